# revision 19
# baseline (speedup 1.0000x reference)
"""GCN layer (CrossViewGCN layer 1) on 8 Trainium2 NeuronCores.

Reference computation (shapes hardcoded):
    X = input[:, :512]                      # [8192, 512]
    A = input[:, 512:8704] + I              # [8192, 8192]
    d = colsum(A); Dh = diag(d^-1/2)
    support = X @ W                         # [8192, 256]
    out_mm  = Dh @ A @ Dh @ support         # [8192, 256]
    return concat(out_mm, A)                # [8192, 8448]

Sharding: 1D row partition of A / output across the 8 cores (1024 rows
each). The diagonal scalings are folded into the small operands and the
bulk matmul is decomposed around its means so the device does a single
fp8 DoubleRow GEMM per core while all mean terms stay exact f32:

    S    = d^-1/2[:, None] * (X @ W)                  (host, [8192, 256])
    A+I  = a0*J + dA,  S = mu + dS   (a0 = 0.5, mu = colmean(S))
    out_mm rows_i = d^-1/2[rows_i] * ( a0*colsum(S)
                                     + rowsum(dA_i)*mu
                                     + dA_i @ dS )    (device: dA_i @ dS)

dA in [-0.5, 0.5] and dS (zero-mean) are an order of magnitude smaller
than A and S, so quantizing them to fp8e4m3 yields ~1e-5 global
relative error (better than a direct bf16 GEMM) while DoubleRow packs a
256-deep contraction per matmul. The device result dA_i @ dS is itself
a small zero-mean correction (the exact mean terms are added on the
host), so it is written back as fp8 as well — 256 KiB instead of 1 MiB
on the wire, at a ~1e-5 global error cost.

Device-side layouts are partition-major ([128, slab, free]) so every
DMA is 128 long contiguous runs — fast HWDGE descriptor generation.
DMA issue alternates between the two HWDGE rings (SP via nc.sync, ACT
via nc.scalar) so transfers on the two rings overlap. Delivery is
deadline-ordered (s-chunks interleaved with a-groups in consumption
order); a-groups are small at the head (PE can start ~2 DMAs in),
1 MiB mid-stream (DMA efficiency), and small at the tail (the PE only
trails the wire by the last group's matmuls).
"""

import numpy as np
import ml_dtypes

NSMP = 8192
NA = 512
DOUT = 256
REALNA = 520
NCORES = 8
ROWS = NSMP // NCORES  # 1024 output rows per core
P = 128
KSLABS = NSMP // P  # 64 contraction slabs of 128
KPAIRS = KSLABS // 2  # 32 DoubleRow slab-pairs (256-deep each)
MM_N = 512  # output free dim per matmul (= one PSUM bank of f32)

A0 = np.float32(0.5)  # mean removed from A+I before fp8 quantization

# PE clock-gate warmup matmuls (~0.48 us each at 1.2 GHz). They serve two
# jobs: keep the PE busy until the first data lands (HAM flips to 2.4 GHz
# ~3.4 us after sustained busy), and buy time for the DMA wire to build a
# buffer ahead of the stream (the wire ramps ~270 -> 400+ GB/s over the
# first ~8 us, while a warm PE consumes ~385 GB/s).
N_WARM = 16
# dS chunks (slab counts): small first so pair 0's stationary lands fast.
S_CHUNKS = [2, 2, 4, 8, 8, 8, 8, 8, 8, 8]  # sum = 64
# dA^T stream groups (slab counts): 0.5 MiB head — smaller heads throttle
# early SDMA parallelism and starve the stream start — 1 MiB steady
# state, fine-grained tail so the last arrival has a short matmul trail.
A_GROUPS = [4, 4, 4, 4, 8, 8, 8, 8, 8, 4, 2, 2]  # sum = 64

_compiled = None
last_results = None  # BassKernelResults of the most recent run (for harnesses)


def _get_compiled():
    global _compiled
    if _compiled is not None:
        return _compiled

    import concourse.bacc as bacc
    import concourse.mybir as mybir
    import concourse.tile as tile

    fp8 = mybir.dt.float8e4
    f32 = mybir.dt.float32
    DR = mybir.MatmulPerfMode.DoubleRow

    nc = bacc.Bacc(
        "TRN2", target_bir_lowering=False, debug=False, num_devices=NCORES
    )
    # partition-major: at[p, t, m] = dA_i^T[t*128 + p, m]
    at = nc.dram_tensor("at", [P, KSLABS, ROWS], fp8, kind="ExternalInput")
    # partition-major: s[p, t, n] = dS[t*128 + p, n]
    s = nc.dram_tensor("s", [P, KSLABS, DOUT], fp8, kind="ExternalInput")
    ot = nc.dram_tensor("ot", [DOUT, ROWS], fp8, kind="ExternalOutput")

    assert sum(S_CHUNKS) == KSLABS and sum(A_GROUPS) == KSLABS
    assert all(c % 2 == 0 for c in S_CHUNKS + A_GROUPS)
    s_offs = np.cumsum([0] + S_CHUNKS)[:-1].tolist()
    a_offs = np.cumsum([0] + A_GROUPS)[:-1].tolist()
    # slab -> (s-chunk index, local slab offset inside the chunk)
    slab2chunk = {}
    for c, (off, sz) in enumerate(zip(s_offs, S_CHUNKS)):
        for t in range(off, off + sz):
            slab2chunk[t] = (c, t - off)

    with tile.TileContext(nc) as tc:
        with (
            tc.tile_pool(name="s_pool", bufs=1) as s_pool,
            tc.tile_pool(name="a_pool", bufs=12) as a_pool,
            tc.tile_pool(name="o_pool", bufs=1) as o_pool,
            tc.tile_pool(name="ps_pool", bufs=1, space="PSUM") as ps_pool,
        ):
            # dS stays resident: chunks totaling [128, 64, 256] fp8 (2 MiB),
            # loaded in consumption order alongside the dA stream.
            s_tiles = [None] * len(S_CHUNKS)

            # OT = dS^T @ dA^T as two [128, 1024] f32 PSUM tiles
            # (n-tile j covers output rows j*128..(j+1)*128 of ot).
            ps = []
            for j in range(DOUT // P):
                ps_t = ps_pool.tile([P, ROWS], f32, name=f"ps{j}", tag=f"ps{j}")
                ps.append(ps_t)

            # Pre-warm the PE HAM clock gate during the preamble/DMA dead
            # time: dependency-free matmuls into a scratch PSUM bank keep
            # the PE busy through one 4096-cycle activity window, so the
            # real stream starts at 2.4 GHz instead of 1.2 GHz. The count
            # is sized so the warmups end ~when the first data DMAs land;
            # more would push the real stream back (PE FIFO is in-order).
            warm_in = o_pool.tile([P, 2, MM_N], fp8, name="warm_in", tag="warm_in")
            warm_ps = ps_pool.tile([P, MM_N], f32, name="warm_ps", tag="warm_ps")
            act_dummy = o_pool.tile([P, 1], fp8, name="act_dummy", tag="act_dummy")
            nc.vector.memset(warm_in[:1, :1, :1], 0.0)
            # trigger the ACT function-table load during the preamble dead
            # zone rather than at the first eviction copy at stream end
            nc.scalar.copy(act_dummy[:1, :1], warm_in[:1, :1, 0:1])
            for _ in range(N_WARM):
                nc.tensor.matmul(
                    warm_ps[:],
                    warm_in[:, :, :P],
                    warm_in[:],
                    start=True,
                    stop=True,
                    perf_mode=DR,
                )

            # Merge S-chunk and dA-group DMAs into one deadline-ordered list
            # (chunk/group starting at slab t is consumed at pair t//2), then
            # alternate rings item-by-item: each ring's FIFO is then also
            # deadline-ordered and the two rings share the wire ~evenly.
            items = []
            si = 0
            for u, aoff in enumerate(a_offs):
                while si < len(S_CHUNKS) and s_offs[si] < aoff + A_GROUPS[u]:
                    items.append(("s", si))
                    si += 1
                items.append(("a", u))
            while si < len(S_CHUNKS):
                items.append(("s", si))
                si += 1

            def mm(q, j, sc, sl, a_t, g):
                # [128, 2, 128] stationary: two k-slabs per PE cell
                lhsT = sc[:, sl : sl + 2, j * P : (j + 1) * P]
                for mc in range(ROWS // MM_N):
                    nc.tensor.matmul(
                        ps[j][:, mc * MM_N : (mc + 1) * MM_N],
                        lhsT,
                        a_t[:, g : g + 2, mc * MM_N : (mc + 1) * MM_N],
                        start=(q == 0),
                        stop=(q == KPAIRS - 1),
                        perf_mode=DR,
                    )

            # ALL data DMAs ride ONE ring (SP) in exact consumption order: a
            # single HWDGE ring's DMA already fans out across all 16 SDMA
            # engines, so a second data ring adds no bandwidth — it only
            # lets a not-yet-needed transfer steal wire from the
            # deadline-critical one (cross-ring completion inversion caused
            # the 2-4 us mid-stream stalls). The ACT ring carries only the
            # tail writebacks.
            last_a = max(u for k, u in items if k == "a")
            for kind, u in items:
                eng = nc.sync
                if kind == "s":
                    off, sz = s_offs[u], S_CHUNKS[u]
                    s_t = s_pool.tile(
                        [P, sz, DOUT], fp8, name=f"s_t{u}", tag=f"s_t{u}"
                    )
                    eng.dma_start(out=s_t[:], in_=s[:, off : off + sz, :])
                    s_tiles[u] = s_t
                    continue
                off, grp = a_offs[u], A_GROUPS[u]
                a_t = a_pool.tile([P, grp, ROWS], fp8, name="a_t", tag="a_t")
                eng.dma_start(out=a_t[:], in_=at[:, off : off + grp, :])
                if u < last_a:
                    for g in range(0, grp, 2):
                        t = off + g  # first slab of the pair
                        c, sl = slab2chunk[t]
                        for j in range(DOUT // P):
                            mm(t // 2, j, s_tiles[c], sl, a_t, g)
                else:
                    # last group j-outer: ps0 finishes all its matmuls one
                    # half-group early, so its eviction overlaps ps1's tail
                    for j in range(DOUT // P):
                        for g in range(0, grp, 2):
                            t = off + g
                            c, sl = slab2chunk[t]
                            mm(t // 2, j, s_tiles[c], sl, a_t, g)

            # evict to fp8 in [128, 512] quarters, vector handling ps0 and
            # scalar ps1 in parallel; each engine writes back what it cast
            for j in range(DOUT // P):
                # ps0 evicts via DVE + SP-ring writebacks, ps1 via ACT +
                # ACT-ring writebacks: two independent evict pipelines (the
                # SP ring's data stream is long done by now)
                ceng, deng = (nc.vector, nc.sync) if j == 0 else (nc.scalar, nc.scalar)
                for mc in range(ROWS // MM_N):
                    o_t = o_pool.tile(
                        [P, MM_N], fp8, name=f"o{j}_{mc}", tag=f"o{j}_{mc}"
                    )
                    if ceng is nc.vector:
                        ceng.tensor_copy(o_t[:], ps[j][:, mc * MM_N : (mc + 1) * MM_N])
                    else:
                        ceng.copy(o_t[:], ps[j][:, mc * MM_N : (mc + 1) * MM_N])
                    deng.dma_start(
                        out=ot[j * P : (j + 1) * P, mc * MM_N : (mc + 1) * MM_N],
                        in_=o_t[:],
                    )

    nc.compile()
    _compiled = nc
    return _compiled


def kernel(input, weight):
    global last_results
    input = np.asarray(input, dtype=np.float32)
    weight = np.asarray(weight, dtype=np.float32)

    X = input[:, :NA]
    A = input[:, REALNA - 8 : REALNA - 8 + NSMP]  # [8192, 8192] view (no +I yet)

    # d = colsum(A + I); the identity adds exactly 1 to every column sum.
    d = A.sum(axis=0, dtype=np.float64) + 1.0
    dinv = (1.0 / np.sqrt(d)).astype(np.float32)  # [8192]
    # rowsum(dA) = rowsum(A + I) - a0*8192, needed for the mean correction
    rowsum_dA = (A.sum(axis=1, dtype=np.float64) + 1.0 - float(A0) * NSMP).astype(
        np.float32
    )

    support = X @ weight  # [8192, 256] f32
    S = support * dinv[:, None]
    mu = S.mean(axis=0, dtype=np.float64).astype(np.float32)  # [256]
    colsum_S = S.sum(axis=0, dtype=np.float64).astype(np.float32)  # [256]
    dS = (S - mu[None, :]).astype(ml_dtypes.float8_e4m3)
    # partition-major [128, 64, 256]
    s_dev = np.ascontiguousarray(dS.reshape(KSLABS, P, DOUT).swapaxes(0, 1))

    diag = np.arange(ROWS)
    in_maps = []
    for i in range(NCORES):
        blk = A[i * ROWS : (i + 1) * ROWS, :]  # [1024, 8192] view
        at_i = (blk.T - A0).astype(ml_dtypes.float8_e4m3)  # [8192, 1024]
        grows = i * ROWS + diag
        # fold the +I into this block's transposed, centered copy
        at_i[grows, diag] = (blk[diag, grows] + (1.0 - A0)).astype(
            ml_dtypes.float8_e4m3
        )
        # partition-major [128, 64, 1024]
        at_dev = np.ascontiguousarray(at_i.reshape(KSLABS, P, ROWS).swapaxes(0, 1))
        in_maps.append({"at": at_dev, "s": s_dev})

    # If BASS_TRACE is set but the axon NTFF hook module is absent, the
    # bass_utils trace path would die on import; provide a no-op hook so it
    # degrades to an untraced run instead.
    try:
        import antenv.axon_hooks  # noqa: F401
    except Exception:
        import sys
        import types

        _m = types.ModuleType("antenv.axon_hooks")
        _m.get_axon_ntff_profile_hook = lambda: None
        _m.set_axon_ntff_profile_hook = lambda h: None
        sys.modules["antenv.axon_hooks"] = _m

    from concourse.bass_utils import run_bass_kernel_spmd

    nc = _get_compiled()
    res = run_bass_kernel_spmd(nc, in_maps, list(range(NCORES)))
    last_results = res

    out = np.empty((NSMP, DOUT + NSMP), dtype=np.float32)
    out[:, DOUT:] = A
    gr = np.arange(NSMP)
    out[gr, DOUT + gr] += 1.0
    # exact mean terms: a0*colsum(S) + rowsum(dA)[:, None] * mu
    mean_terms = float(A0) * colsum_S[None, :] + rowsum_dA[:, None] * mu[None, :]
    for i in range(NCORES):
        # [256, 1024] fp8 = (dA_i @ dS)^T quantized on-device
        ot_i = np.asarray(res.results[i]["ot"]).astype(np.float32)
        rows = slice(i * ROWS, (i + 1) * ROWS)
        out[rows, :DOUT] = (ot_i.T + mean_terms[rows]) * dinv[rows, None]
    return out


# revision 20
# speedup vs baseline: 1.0258x; 1.0258x over previous
"""GCN layer (CrossViewGCN layer 1) on 8 Trainium2 NeuronCores.

Reference computation (shapes hardcoded):
    X = input[:, :512]                      # [8192, 512]
    A = input[:, 512:8704] + I              # [8192, 8192]
    d = colsum(A); Dh = diag(d^-1/2)
    support = X @ W                         # [8192, 256]
    out_mm  = Dh @ A @ Dh @ support         # [8192, 256]
    return concat(out_mm, A)                # [8192, 8448]

Sharding: 1D row partition of A / output across the 8 cores (1024 rows
each). The diagonal scalings are folded into the small operands and the
bulk matmul is decomposed around its means so the device does a single
fp8 DoubleRow GEMM per core while all mean terms stay exact f32:

    S    = d^-1/2[:, None] * (X @ W)                  (host, [8192, 256])
    A+I  = a0*J + dA,  S = mu + dS   (a0 = 0.5, mu = colmean(S))
    out_mm rows_i = d^-1/2[rows_i] * ( a0*colsum(S)
                                     + rowsum(dA_i)*mu
                                     + dA_i @ dS )    (device: dA_i @ dS)

dA in [-0.5, 0.5] and dS (zero-mean) are an order of magnitude smaller
than A and S, so quantizing them to fp8e4m3 yields ~1e-5 global
relative error (better than a direct bf16 GEMM) while DoubleRow packs a
256-deep contraction per matmul. The device result dA_i @ dS is itself
a small zero-mean correction (the exact mean terms are added on the
host), so it is written back as fp8 as well — 256 KiB instead of 1 MiB
on the wire, at a ~1e-5 global error cost.

Device-side layouts are partition-major ([128, slab, free]) so every
DMA is 128 long contiguous runs — fast HWDGE descriptor generation.
DMA issue alternates between the two HWDGE rings (SP via nc.sync, ACT
via nc.scalar) so transfers on the two rings overlap. Delivery is
deadline-ordered (s-chunks interleaved with a-groups in consumption
order); a-groups are small at the head (PE can start ~2 DMAs in),
1 MiB mid-stream (DMA efficiency), and small at the tail (the PE only
trails the wire by the last group's matmuls).
"""

import numpy as np
import ml_dtypes

NSMP = 8192
NA = 512
DOUT = 256
REALNA = 520
NCORES = 8
ROWS = NSMP // NCORES  # 1024 output rows per core
P = 128
KSLABS = NSMP // P  # 64 contraction slabs of 128
KPAIRS = KSLABS // 2  # 32 DoubleRow slab-pairs (256-deep each)
MM_N = 512  # output free dim per matmul (= one PSUM bank of f32)

A0 = np.float32(0.5)  # mean removed from A+I before fp8 quantization

# PE clock-gate warmup matmuls (~0.48 us each at 1.2 GHz). They serve two
# jobs: keep the PE busy until the first data lands (HAM flips to 2.4 GHz
# ~3.4 us after sustained busy), and buy time for the DMA wire to build a
# buffer ahead of the stream (the wire ramps ~270 -> 400+ GB/s over the
# first ~8 us, while a warm PE consumes ~385 GB/s).
N_WARM = 10
# dS chunks (slab counts): small first so pair 0's stationary lands fast.
S_CHUNKS = [2, 2, 4, 8, 8, 8, 8, 8, 8, 8]  # sum = 64
# dA^T stream groups (slab counts): 0.5 MiB head — smaller heads throttle
# early SDMA parallelism and starve the stream start — 1 MiB steady
# state, fine-grained tail so the last arrival has a short matmul trail.
A_GROUPS = [4, 4, 4, 4, 8, 8, 8, 8, 8, 8]  # sum = 64

_compiled = None
last_results = None  # BassKernelResults of the most recent run (for harnesses)


def _get_compiled():
    global _compiled
    if _compiled is not None:
        return _compiled

    import concourse.bacc as bacc
    import concourse.mybir as mybir
    import concourse.tile as tile

    fp8 = mybir.dt.float8e4
    f32 = mybir.dt.float32
    DR = mybir.MatmulPerfMode.DoubleRow

    nc = bacc.Bacc(
        "TRN2", target_bir_lowering=False, debug=False, num_devices=NCORES
    )
    # partition-major: at[p, t, m] = dA_i^T[t*128 + p, m]
    at = nc.dram_tensor("at", [P, KSLABS, ROWS], fp8, kind="ExternalInput")
    # partition-major: s[p, t, n] = dS[t*128 + p, n]
    s = nc.dram_tensor("s", [P, KSLABS, DOUT], fp8, kind="ExternalInput")
    ot = nc.dram_tensor("ot", [DOUT, ROWS], fp8, kind="ExternalOutput")

    assert sum(S_CHUNKS) == KSLABS and sum(A_GROUPS) == KSLABS
    assert all(c % 2 == 0 for c in S_CHUNKS + A_GROUPS)
    s_offs = np.cumsum([0] + S_CHUNKS)[:-1].tolist()
    a_offs = np.cumsum([0] + A_GROUPS)[:-1].tolist()
    # slab -> (s-chunk index, local slab offset inside the chunk)
    slab2chunk = {}
    for c, (off, sz) in enumerate(zip(s_offs, S_CHUNKS)):
        for t in range(off, off + sz):
            slab2chunk[t] = (c, t - off)

    with tile.TileContext(nc) as tc:
        with (
            tc.tile_pool(name="s_pool", bufs=1) as s_pool,
            tc.tile_pool(name="a_pool", bufs=5) as a_pool,
            tc.tile_pool(name="o_pool", bufs=1) as o_pool,
            tc.tile_pool(name="ps_pool", bufs=1, space="PSUM") as ps_pool,
        ):
            # dS stays resident: chunks totaling [128, 64, 256] fp8 (2 MiB),
            # loaded in consumption order alongside the dA stream.
            s_tiles = [None] * len(S_CHUNKS)

            # OT = dS^T @ dA^T as two [128, 1024] f32 PSUM tiles
            # (n-tile j covers output rows j*128..(j+1)*128 of ot).
            ps = []
            for j in range(DOUT // P):
                ps_t = ps_pool.tile([P, ROWS], f32, name=f"ps{j}", tag=f"ps{j}")
                ps.append(ps_t)

            # Pre-warm the PE HAM clock gate during the preamble/DMA dead
            # time: dependency-free matmuls into a scratch PSUM bank keep
            # the PE busy through one 4096-cycle activity window, so the
            # real stream starts at 2.4 GHz instead of 1.2 GHz. The count
            # is sized so the warmups end ~when the first data DMAs land;
            # more would push the real stream back (PE FIFO is in-order).
            warm_in = o_pool.tile([P, 2, MM_N], fp8, name="warm_in", tag="warm_in")
            warm_ps = ps_pool.tile([P, MM_N], f32, name="warm_ps", tag="warm_ps")
            act_dummy = o_pool.tile([P, 1], fp8, name="act_dummy", tag="act_dummy")
            nc.vector.memset(warm_in[:1, :1, :1], 0.0)
            # trigger the ACT function-table load during the preamble dead
            # zone rather than at the first eviction copy at stream end
            nc.scalar.copy(act_dummy[:1, :1], warm_in[:1, :1, 0:1])
            for _ in range(N_WARM):
                nc.tensor.matmul(
                    warm_ps[:],
                    warm_in[:, :, :P],
                    warm_in[:],
                    start=True,
                    stop=True,
                    perf_mode=DR,
                )

            # Merge S-chunk and dA-group DMAs into one deadline-ordered list
            # (chunk/group starting at slab t is consumed at pair t//2), then
            # alternate rings item-by-item: each ring's FIFO is then also
            # deadline-ordered and the two rings share the wire ~evenly.
            items = []
            si = 0
            for u, aoff in enumerate(a_offs):
                while si < len(S_CHUNKS) and s_offs[si] < aoff + A_GROUPS[u]:
                    items.append(("s", si))
                    si += 1
                items.append(("a", u))
            while si < len(S_CHUNKS):
                items.append(("s", si))
                si += 1

            def mm(q, j, sc, sl, a_t, g):
                # [128, 2, 128] stationary: two k-slabs per PE cell
                lhsT = sc[:, sl : sl + 2, j * P : (j + 1) * P]
                for mc in range(ROWS // MM_N):
                    nc.tensor.matmul(
                        ps[j][:, mc * MM_N : (mc + 1) * MM_N],
                        lhsT,
                        a_t[:, g : g + 2, mc * MM_N : (mc + 1) * MM_N],
                        start=(q == 0),
                        stop=(q == KPAIRS - 1),
                        perf_mode=DR,
                    )

            # Item-alternate the two HWDGE rings: each ring's FIFO stays
            # deadline-ordered and the per-ring descriptor backlog is
            # halved. A DMA's completion semaphore fires with its SLOWEST
            # SDMA engine's share, and that tail grows with queue backlog
            # (measured 4-6 us of sem-lag with everything on one deep ring)
            # — so backlog is also bounded by the a_pool WAR pacing (bufs).
            last_a = max(u for k, u in items if k == "a")
            for idx, (kind, u) in enumerate(items):
                eng = nc.sync if idx % 2 == 0 else nc.scalar
                if kind == "s":
                    off, sz = s_offs[u], S_CHUNKS[u]
                    s_t = s_pool.tile(
                        [P, sz, DOUT], fp8, name=f"s_t{u}", tag=f"s_t{u}"
                    )
                    eng.dma_start(out=s_t[:], in_=s[:, off : off + sz, :])
                    s_tiles[u] = s_t
                    continue
                off, grp = a_offs[u], A_GROUPS[u]
                a_t = a_pool.tile([P, grp, ROWS], fp8, name="a_t", tag="a_t")
                eng.dma_start(out=a_t[:], in_=at[:, off : off + grp, :])
                if u < last_a:
                    for g in range(0, grp, 2):
                        t = off + g  # first slab of the pair
                        c, sl = slab2chunk[t]
                        for j in range(DOUT // P):
                            mm(t // 2, j, s_tiles[c], sl, a_t, g)
                else:
                    # last group j-outer: ps0 finishes all its matmuls one
                    # half-group early, so its eviction overlaps ps1's tail
                    for j in range(DOUT // P):
                        for g in range(0, grp, 2):
                            t = off + g
                            c, sl = slab2chunk[t]
                            mm(t // 2, j, s_tiles[c], sl, a_t, g)

            # evict to fp8 in [128, 512] quarters, vector handling ps0 and
            # scalar ps1 in parallel; each engine writes back what it cast
            for j in range(DOUT // P):
                # ps0 evicts via DVE + SP-ring writebacks, ps1 via ACT +
                # ACT-ring writebacks: two independent evict pipelines (the
                # SP ring's data stream is long done by now)
                ceng, deng = (nc.vector, nc.sync) if j == 0 else (nc.scalar, nc.scalar)
                for mc in range(ROWS // MM_N):
                    o_t = o_pool.tile(
                        [P, MM_N], fp8, name=f"o{j}_{mc}", tag=f"o{j}_{mc}"
                    )
                    if ceng is nc.vector:
                        ceng.tensor_copy(o_t[:], ps[j][:, mc * MM_N : (mc + 1) * MM_N])
                    else:
                        ceng.copy(o_t[:], ps[j][:, mc * MM_N : (mc + 1) * MM_N])
                    deng.dma_start(
                        out=ot[j * P : (j + 1) * P, mc * MM_N : (mc + 1) * MM_N],
                        in_=o_t[:],
                    )

    nc.compile()
    _compiled = nc
    return _compiled


def kernel(input, weight):
    global last_results
    input = np.asarray(input, dtype=np.float32)
    weight = np.asarray(weight, dtype=np.float32)

    X = input[:, :NA]
    A = input[:, REALNA - 8 : REALNA - 8 + NSMP]  # [8192, 8192] view (no +I yet)

    # d = colsum(A + I); the identity adds exactly 1 to every column sum.
    d = A.sum(axis=0, dtype=np.float64) + 1.0
    dinv = (1.0 / np.sqrt(d)).astype(np.float32)  # [8192]
    # rowsum(dA) = rowsum(A + I) - a0*8192, needed for the mean correction
    rowsum_dA = (A.sum(axis=1, dtype=np.float64) + 1.0 - float(A0) * NSMP).astype(
        np.float32
    )

    support = X @ weight  # [8192, 256] f32
    S = support * dinv[:, None]
    mu = S.mean(axis=0, dtype=np.float64).astype(np.float32)  # [256]
    colsum_S = S.sum(axis=0, dtype=np.float64).astype(np.float32)  # [256]
    dS = (S - mu[None, :]).astype(ml_dtypes.float8_e4m3)
    # partition-major [128, 64, 256]
    s_dev = np.ascontiguousarray(dS.reshape(KSLABS, P, DOUT).swapaxes(0, 1))

    diag = np.arange(ROWS)
    in_maps = []
    for i in range(NCORES):
        blk = A[i * ROWS : (i + 1) * ROWS, :]  # [1024, 8192] view
        at_i = (blk.T - A0).astype(ml_dtypes.float8_e4m3)  # [8192, 1024]
        grows = i * ROWS + diag
        # fold the +I into this block's transposed, centered copy
        at_i[grows, diag] = (blk[diag, grows] + (1.0 - A0)).astype(
            ml_dtypes.float8_e4m3
        )
        # partition-major [128, 64, 1024]
        at_dev = np.ascontiguousarray(at_i.reshape(KSLABS, P, ROWS).swapaxes(0, 1))
        in_maps.append({"at": at_dev, "s": s_dev})

    # If BASS_TRACE is set but the axon NTFF hook module is absent, the
    # bass_utils trace path would die on import; provide a no-op hook so it
    # degrades to an untraced run instead.
    try:
        import antenv.axon_hooks  # noqa: F401
    except Exception:
        import sys
        import types

        _m = types.ModuleType("antenv.axon_hooks")
        _m.get_axon_ntff_profile_hook = lambda: None
        _m.set_axon_ntff_profile_hook = lambda h: None
        sys.modules["antenv.axon_hooks"] = _m

    from concourse.bass_utils import run_bass_kernel_spmd

    nc = _get_compiled()
    res = run_bass_kernel_spmd(nc, in_maps, list(range(NCORES)))
    last_results = res

    out = np.empty((NSMP, DOUT + NSMP), dtype=np.float32)
    out[:, DOUT:] = A
    gr = np.arange(NSMP)
    out[gr, DOUT + gr] += 1.0
    # exact mean terms: a0*colsum(S) + rowsum(dA)[:, None] * mu
    mean_terms = float(A0) * colsum_S[None, :] + rowsum_dA[:, None] * mu[None, :]
    for i in range(NCORES):
        # [256, 1024] fp8 = (dA_i @ dS)^T quantized on-device
        ot_i = np.asarray(res.results[i]["ot"]).astype(np.float32)
        rows = slice(i * ROWS, (i + 1) * ROWS)
        out[rows, :DOUT] = (ot_i.T + mean_terms[rows]) * dinv[rows, None]
    return out
